# revision 8
# baseline (speedup 1.0000x reference)
"""RNN-T joint network kernel for 8 Trainium2 NeuronCores.

out[b,t,u,:] = W2 @ tanh(W1e @ enc[b,t] + W1d @ dec[b,u] + b1) + b2

Shapes: B=4, T=200, U=100, D=512, H=1024, O=512 (fp32 in/out).
Sharding: T split 8 ways (25 t's per core); dec + weights replicated.

All matmul inputs are bf16 (rel-err budget 2e-2; measured bf16 error
~3e-3; fp8 measured 3.4e-2 — over budget). bf16/fp32r both stream at
1 cycle/row on the PE, so bf16's win is half the DMA/SBUF traffic and
no fp32r cast instructions at startup.

Input DMA is descriptor-dispatch-bound (~20ns/descriptor), so inputs
are packed host-side into two wide dram tensors (one per HWDGE ring)
giving 9-11KB contiguous per-partition lines: 128 descriptors per
transfer instead of ~1000 total.

Per-core device program:
  Phase 1: ench[k][h,100] = W1e@encT + b1, dech[k][h,400] = W1d@decT,
           k-chunks split into A (k=0..5, DVE-owned) and B (k=6..7,
           GpSimd-owned) tile groups.
  Phase 2: per chunk (b, up to 5 t's -> <=500 rows):
           s = ench (+) dech fused broadcast-add with 4D APs (DVE does
           the A half; GpSimd, which measures ~3.5ns/elem, gets only
           the 2-chunk B half), in-place tanh per half (ACT), then per
           oc in 0..3 an 8-matmul PSUM accumulation group against W2
           (A-half groups first, B closes them), psum->sbuf copy
           (ACT oc0/oc1, DVE oc2/oc3 — GpSimd cannot read PSUM), DMA
           out on alternating rings.
  b2 is added on the host.
"""

from contextlib import ExitStack

import ml_dtypes
import numpy as np

import concourse.bacc as bacc
import concourse.bass as bass
import concourse.mybir as mybir
import concourse.tile as tile
from concourse.bass_utils import run_bass_kernel_spmd

F32 = mybir.dt.float32
BF16 = mybir.dt.bfloat16

B, T, U, D, H, O = 4, 200, 100, 512, 1024, 512
NCORES = 8
TLOC = T // NCORES            # 25 t's per core
PAIRS = B * TLOC              # 100 (b,t) pairs per core
BU = B * U                    # 400
ROWS = PAIRS * U              # 10000 output rows per core
DK = D // 128                 # 4 contraction chunks for phase 1
HK = H // 128                 # 8 h chunks
KA = 6                        # k chunks in the A half (k=0..5)
KB = HK - KA                  # 2 k chunks in the B half (k=6..7)
TCH = 5                       # max t's per phase-2 chunk
CHMAX = TCH * U               # 500 rows max per chunk

# packed input layouts (bf16 cols)
ENC_W = DK * PAIRS            # 400
W1_W = DK * H                 # 4096
DEC_W = DK * BU               # 1600
INS_W = ENC_W + W1_W          # sync ring: encT | w1eT
INC_W = DEC_W + W1_W          # scalar ring: decT | w1dT

_CACHE = {}


def _chunks():
    sizes_by_b = [
        [2, 4, 5, 5, 5, 4],
        [5] * 5,
        [5] * 5,
        [5, 5, 5, 5, 4, 1],
    ]
    out = []
    for b, sizes in enumerate(sizes_by_b):
        t0 = 0
        for tch in sizes:
            out.append((b, t0, tch))
            t0 += tch
        assert t0 == TLOC
    return out


def _build():
    nc = bacc.Bacc("TRN2", target_bir_lowering=False, debug=False,
                   num_devices=NCORES)
    inS = nc.dram_tensor("inS", [128, INS_W], BF16, kind="ExternalInput")
    inC = nc.dram_tensor("inC", [128, INC_W], BF16, kind="ExternalInput")
    w2T = nc.dram_tensor("w2T", [128, HK * O], BF16, kind="ExternalInput")
    b1r = nc.dram_tensor("b1r", [128, HK], F32, kind="ExternalInput")
    out = nc.dram_tensor("out", [O, ROWS], F32, kind="ExternalOutput")

    with tile.TileContext(nc) as tc, ExitStack() as ctx:
        consts = ctx.enter_context(tc.tile_pool(name="consts", bufs=1))
        spoolA = ctx.enter_context(tc.tile_pool(name="spoolA", bufs=4))
        spoolB = ctx.enter_context(tc.tile_pool(name="spoolB", bufs=4))
        opool = ctx.enter_context(tc.tile_pool(name="opool", bufs=8))
        psB = ctx.enter_context(tc.tile_pool(name="psB", bufs=8, space="PSUM"))

        inS_s = consts.tile([128, INS_W], BF16)
        inC_s = consts.tile([128, INC_W], BF16)
        w2_s = consts.tile([128, HK * O], BF16)
        b1_s = consts.tile([128, HK], F32)
        nc.sync.dma_start(inS_s[:], inS[:])
        nc.scalar.dma_start(inC_s[:], inC[:])
        nc.sync.dma_start(w2_s[:], w2T[:])
        nc.scalar.dma_start(b1_s[:], b1r[:])
        encT_s = inS_s[:, :ENC_W]
        w1e_s = inS_s[:, ENC_W:]
        decT_s = inC_s[:, :DEC_W]
        w1d_s = inC_s[:, DEC_W:]

        # ---- phase 1 ----
        ench_t = {"A": consts.tile([128, KA * PAIRS], BF16, name="enchA"),
                  "B": consts.tile([128, KB * PAIRS], BF16, name="enchB")}
        dech_t = {"A": consts.tile([128, KA * BU], BF16, name="dechA"),
                  "B": consts.tile([128, KB * BU], BF16, name="dechB")}

        def halfslot(k):
            return ("A", k) if k < KA else ("B", k - KA)

        for k in range(HK):
            pe = psB.tile([128, 512], F32, tag="psB", name="pe")[:, :PAIRS]
            for dk in range(DK):
                nc.tensor.matmul(
                    pe[:],
                    lhsT=w1e_s[:, dk * H + k * 128: dk * H + (k + 1) * 128],
                    rhs=encT_s[:, dk * PAIRS:(dk + 1) * PAIRS],
                    start=(dk == 0), stop=(dk == DK - 1),
                )
            hf, kk = halfslot(k)
            nc.vector.tensor_scalar_add(
                ench_t[hf][:, kk * PAIRS:(kk + 1) * PAIRS], pe[:],
                b1_s[:, k:k + 1])
        for k in range(HK):
            pd = psB.tile([128, 512], F32, tag="psB", name="pd")[:, :BU]
            for dk in range(DK):
                nc.tensor.matmul(
                    pd[:],
                    lhsT=w1d_s[:, dk * H + k * 128: dk * H + (k + 1) * 128],
                    rhs=decT_s[:, dk * BU:(dk + 1) * BU],
                    start=(dk == 0), stop=(dk == DK - 1),
                )
            hf, kk = halfslot(k)
            dst = dech_t[hf][:, kk * BU:(kk + 1) * BU]
            if k % 2 == 0:
                nc.vector.tensor_copy(dst, pd[:])
            else:
                nc.scalar.activation(dst, pd[:],
                                     mybir.ActivationFunctionType.Copy)

        # ---- phase 2 ----
        for b, t0c, tch in _chunks():
            rows_c = tch * U
            row0 = b * (TLOC * U) + t0c * U

            s_t = {"A": spoolA.tile([128, KA * CHMAX], BF16, tag="sA",
                                    name="sA"),
                   "B": spoolB.tile([128, KB * CHMAX], BF16, tag="sB",
                                    name="sB")}
            for hf, nk, eng in (("A", KA, nc.vector), ("B", KB, nc.gpsimd)):
                # fused broadcast add over (k, t, u) with 4D APs
                dech_ap = dech_t[hf][:].rearrange(
                    "p (k bu) -> p k bu", k=nk)[:, :, b * U:(b + 1) * U]
                dech_ap = dech_ap.rearrange("p k (a u) -> p k a u", a=1)
                c0 = b * TLOC + t0c
                ench_ap = ench_t[hf][:].rearrange(
                    "p (k c) -> p k c", k=nk)[:, :, c0:c0 + tch]
                ench_ap = ench_ap.rearrange("p k (t a) -> p k t a", a=1)
                bc_d, bc_e = bass.broadcast_tensor_aps(dech_ap, ench_ap)
                outap = s_t[hf][:, :nk * rows_c].rearrange(
                    "p (k t u) -> p k t u", k=nk, t=tch)
                eng.tensor_tensor(outap, bc_d, bc_e, mybir.AluOpType.add)
                nc.scalar.activation(s_t[hf][:, :nk * rows_c],
                                     s_t[hf][:, :nk * rows_c],
                                     mybir.ActivationFunctionType.Tanh)

            # A-half accumulation groups first (start), B-half closes
            # them (stop)
            ps = []
            for oc in range(O // 128):
                p = psB.tile([128, 512], F32, tag="psB", name="p")[:, :rows_c]
                ps.append(p)
                for k in range(KA):
                    nc.tensor.matmul(
                        p[:],
                        lhsT=w2_s[:, k * O + oc * 128: k * O + (oc + 1) * 128],
                        rhs=s_t["A"][:, k * rows_c:(k + 1) * rows_c],
                        start=(k == 0), stop=False,
                    )
            for oc in range(O // 128):
                for kk in range(KB):
                    k = KA + kk
                    nc.tensor.matmul(
                        ps[oc][:],
                        lhsT=w2_s[:, k * O + oc * 128: k * O + (oc + 1) * 128],
                        rhs=s_t["B"][:, kk * rows_c:(kk + 1) * rows_c],
                        start=False, stop=(kk == KB - 1),
                    )
            for oc in range(O // 128):
                ot = opool.tile([128, CHMAX], F32, tag="ot",
                                name="ot")[:, :rows_c]
                # gpsimd cannot access PSUM; split copies ACT/DVE
                if oc < 2:
                    nc.scalar.activation(ot[:], ps[oc][:],
                                         mybir.ActivationFunctionType.Copy)
                else:
                    nc.vector.tensor_copy(ot[:], ps[oc][:])
                ring = nc.sync if oc % 2 == 0 else nc.scalar
                ring.dma_start(
                    out[oc * 128:(oc + 1) * 128, row0:row0 + rows_c], ot[:])
    nc.compile()
    return nc


def _chunk128(a):
    # [n*128, w] -> [128, n*w]: partition p holds row k*128+p of chunk k
    n = a.shape[0] // 128
    return np.ascontiguousarray(
        a.reshape(n, 128, a.shape[1]).transpose(1, 0, 2).reshape(128, -1))


def _bf16(a):
    return np.ascontiguousarray(a).astype(ml_dtypes.bfloat16)


def kernel(enc_state, dec_state, W1, b1, W2, b2, _trace=False):
    enc_state = np.ascontiguousarray(enc_state, dtype=np.float32)
    dec_state = np.ascontiguousarray(dec_state, dtype=np.float32)
    W1 = np.asarray(W1, dtype=np.float32)
    b1 = np.asarray(b1, dtype=np.float32)
    W2 = np.asarray(W2, dtype=np.float32)
    b2 = np.asarray(b2, dtype=np.float32)

    if "nc" not in _CACHE:
        _CACHE["nc"] = _build()
    nc = _CACHE["nc"]

    decT = _chunk128(dec_state.reshape(B * U, D).T)
    w1eT = _chunk128(W1[:, :D].T)
    w1dT = _chunk128(W1[:, D:].T)
    w2T = _bf16(_chunk128(W2.T))
    b1r = np.ascontiguousarray(b1.reshape(HK, 128).T)
    inC = _bf16(np.concatenate([decT, w1dT], axis=1))

    in_maps = []
    for c in range(NCORES):
        enc_c = enc_state[:, c * TLOC:(c + 1) * TLOC, :].reshape(PAIRS, D)
        encT = _chunk128(enc_c.T)
        in_maps.append({
            "inS": _bf16(np.concatenate([encT, w1eT], axis=1)),
            "inC": inC, "w2T": w2T, "b1r": b1r,
        })

    res = run_bass_kernel_spmd(nc, in_maps, list(range(NCORES)), trace=_trace)
    out = np.empty((B, T, U, O), dtype=np.float32)
    for c in range(NCORES):
        out[:, c * TLOC:(c + 1) * TLOC] = (
            res.results[c]["out"].T.reshape(B, TLOC, U, O))
    out += b2
    if _trace:
        kernel.last_results = res
    return out


# revision 16
# speedup vs baseline: 1.0385x; 1.0385x over previous
"""RNN-T joint network kernel for 8 Trainium2 NeuronCores.

out[b,t,u,:] = W2 @ tanh(W1e @ enc[b,t] + W1d @ dec[b,u] + b1) + b2

Shapes: B=4, T=200, U=100, D=512, H=1024, O=512 (fp32 in/out).
Sharding: T split 8 ways (25 t's per core); dec + weights replicated.

All matmul inputs are bf16 (rel-err budget 2e-2; measured bf16 error
~3e-3; fp8 measured 3.4e-2 — over budget). bf16/fp32r both stream at
1 cycle/row on the PE, so bf16's win is half the DMA/SBUF traffic and
no fp32r cast instructions at startup.

Input DMA is descriptor-dispatch-bound (~20ns/descriptor), so inputs
are packed host-side into two wide dram tensors (one per HWDGE ring)
giving 9-11KB contiguous per-partition lines.

Phase 2 emission is software-pipelined by one chunk — build(i+1)
[broadcast-add + tanh], then matmuls(i), then psum copies(i) — so the
in-order DVE/ACT queues always hold ready work ahead of the
PE-dependent psum copies (avoids head-of-line blocking stalls).
Chunks are up to 5 t's (500-row matmul streams; the matmul moving
size is ISA-limited to 512 rows = one PSUM bank).

Engine assignment per chunk: DVE broadcast-adds k0..6 (GpSimd costs
~0.35us per 100-elem line, so it only gets k7), ACT does both tanhs
and 2 psum copies, DVE the other 2 copies (GpSimd cannot read PSUM).
b2 is added on the host.
"""

from contextlib import ExitStack

import ml_dtypes
import numpy as np

import concourse.bacc as bacc
import concourse.bass as bass
import concourse.mybir as mybir
import concourse.tile as tile
from concourse.bass_utils import run_bass_kernel_spmd

F32 = mybir.dt.float32
BF16 = mybir.dt.bfloat16

B, T, U, D, H, O = 4, 200, 100, 512, 1024, 512
NCORES = 8
TLOC = T // NCORES            # 25 t's per core
PAIRS = B * TLOC              # 100 (b,t) pairs per core
BU = B * U                    # 400
ROWS = PAIRS * U              # 10000 output rows per core
DK = D // 128                 # 4 contraction chunks for phase 1
HK = H // 128                 # 8 h chunks
KA = 7                        # k chunks in the A half (k=0..6, DVE)
KB = HK - KA                  # 1 k chunk in the B half (k=7, GpSimd)
CHMAX = 500                   # max rows per phase-2 chunk (matmul moving
                              # size is ISA-limited to 512 = one PSUM bank)

ENC_W = DK * PAIRS            # 400
W1_W = DK * H                 # 4096
DEC_W = DK * BU               # 1600
INS_W = ENC_W + W1_W          # sync ring: encT | w1eT
INC_W = DEC_W + W1_W          # scalar ring: decT | w1dT

_CACHE = {}


def _chunks():
    sizes_by_b = [
        [2, 4, 5, 5, 5, 4],
        [5] * 5,
        [5] * 5,
        [5, 5, 5, 5, 4, 1],
    ]
    out = []
    for b, sizes in enumerate(sizes_by_b):
        t0 = 0
        for tch in sizes:
            out.append((b, t0, tch))
            t0 += tch
        assert t0 == TLOC
    return out


def _build():
    nc = bacc.Bacc("TRN2", target_bir_lowering=False, debug=False,
                   num_devices=NCORES)
    inS = nc.dram_tensor("inS", [128, INS_W], BF16, kind="ExternalInput")
    inC = nc.dram_tensor("inC", [128, INC_W], BF16, kind="ExternalInput")
    w2T = nc.dram_tensor("w2T", [128, HK * O], BF16, kind="ExternalInput")
    b1r = nc.dram_tensor("b1r", [128, HK], F32, kind="ExternalInput")
    out = nc.dram_tensor("out", [O, ROWS], F32, kind="ExternalOutput")

    with tile.TileContext(nc) as tc, ExitStack() as ctx:
        consts = ctx.enter_context(tc.tile_pool(name="consts", bufs=1))
        spoolA = ctx.enter_context(tc.tile_pool(name="spoolA", bufs=3))
        spoolB = ctx.enter_context(tc.tile_pool(name="spoolB", bufs=3))
        opool = ctx.enter_context(tc.tile_pool(name="opool", bufs=8))
        psB = ctx.enter_context(tc.tile_pool(name="psB", bufs=8, space="PSUM"))

        inS_s = consts.tile([128, INS_W], BF16)
        inC_s = consts.tile([128, INC_W], BF16)
        w2_s = consts.tile([128, HK * O], BF16)
        b1_s = consts.tile([128, HK], F32)
        nc.sync.dma_start(inS_s[:], inS[:])
        nc.scalar.dma_start(inC_s[:], inC[:])
        nc.sync.dma_start(w2_s[:], w2T[:])
        nc.scalar.dma_start(b1_s[:], b1r[:])
        encT_s = inS_s[:, :ENC_W]
        w1e_s = inS_s[:, ENC_W:]
        decT_s = inC_s[:, :DEC_W]
        w1d_s = inC_s[:, DEC_W:]

        # ---- phase 1 ----
        ench_t = {"A": consts.tile([128, KA * PAIRS], BF16, name="enchA"),
                  "B": consts.tile([128, KB * PAIRS], BF16, name="enchB")}
        dech_t = {"A": consts.tile([128, KA * BU], BF16, name="dechA"),
                  "B": consts.tile([128, KB * BU], BF16, name="dechB")}

        def halfslot(k):
            return ("A", k) if k < KA else ("B", k - KA)

        for k in range(HK):
            pe = psB.tile([128, 512], F32, tag="psB", name="pe")[:, :PAIRS]
            for dk in range(DK):
                nc.tensor.matmul(
                    pe[:],
                    lhsT=w1e_s[:, dk * H + k * 128: dk * H + (k + 1) * 128],
                    rhs=encT_s[:, dk * PAIRS:(dk + 1) * PAIRS],
                    start=(dk == 0), stop=(dk == DK - 1),
                )
            hf, kk = halfslot(k)
            # ench copies on ACT (Identity+bias folds the b1 add) so the
            # DVE queue is free for the first chunk's broadcast-add
            nc.scalar.activation(
                ench_t[hf][:, kk * PAIRS:(kk + 1) * PAIRS], pe[:],
                mybir.ActivationFunctionType.Identity, bias=b1_s[:, k:k + 1])
        for k in range(HK):
            pd = psB.tile([128, 512], F32, tag="psB", name="pd")[:, :BU]
            for dk in range(DK):
                nc.tensor.matmul(
                    pd[:],
                    lhsT=w1d_s[:, dk * H + k * 128: dk * H + (k + 1) * 128],
                    rhs=decT_s[:, dk * BU:(dk + 1) * BU],
                    start=(dk == 0), stop=(dk == DK - 1),
                )
            hf, kk = halfslot(k)
            dst = dech_t[hf][:, kk * BU:(kk + 1) * BU]
            if k % 2 == 0:
                nc.vector.tensor_copy(dst, pd[:])
            else:
                nc.scalar.activation(dst, pd[:],
                                     mybir.ActivationFunctionType.Copy)

        # ---- phase 2, software-pipelined by one chunk ----
        chunks = _chunks()
        s_tiles = [None] * len(chunks)
        ps_tiles = [None] * len(chunks)

        def build(i):
            b, t0c, tch = chunks[i]
            rows_c = tch * U
            s_t = {"A": spoolA.tile([128, KA * CHMAX], BF16, tag="sA",
                                    name="sA"),
                   "B": spoolB.tile([128, KB * CHMAX], BF16, tag="sB",
                                    name="sB")}
            s_tiles[i] = s_t
            for hf, nk, eng in (("A", KA, nc.vector), ("B", KB, nc.gpsimd)):
                dech_ap = dech_t[hf][:].rearrange(
                    "p (k bu) -> p k bu", k=nk)[:, :, b * U:(b + 1) * U]
                dech_ap = dech_ap.rearrange("p k (a u) -> p k a u", a=1)
                c0 = b * TLOC + t0c
                ench_ap = ench_t[hf][:].rearrange(
                    "p (k c) -> p k c", k=nk)[:, :, c0:c0 + tch]
                ench_ap = ench_ap.rearrange("p k (t a) -> p k t a", a=1)
                bc_d, bc_e = bass.broadcast_tensor_aps(dech_ap, ench_ap)
                outap = s_t[hf][:, :nk * rows_c].rearrange(
                    "p (k t u) -> p k t u", k=nk, t=tch)
                eng.tensor_tensor(outap, bc_d, bc_e, mybir.AluOpType.add)
                nc.scalar.activation(s_t[hf][:, :nk * rows_c],
                                     s_t[hf][:, :nk * rows_c],
                                     mybir.ActivationFunctionType.Tanh)

        def mms(i):
            b, t0c, tch = chunks[i]
            rows_c = tch * U
            s_t = s_tiles[i]
            ps = []
            for oc in range(O // 128):
                p = psB.tile([128, 512], F32, tag="psB",
                             name="p")[:, :rows_c]
                ps.append(p)
                for k in range(KA):
                    nc.tensor.matmul(
                        p[:],
                        lhsT=w2_s[:, k * O + oc * 128: k * O + (oc + 1) * 128],
                        rhs=s_t["A"][:, k * rows_c:(k + 1) * rows_c],
                        start=(k == 0), stop=False,
                    )
                k = KA
                nc.tensor.matmul(
                    p[:],
                    lhsT=w2_s[:, k * O + oc * 128: k * O + (oc + 1) * 128],
                    rhs=s_t["B"][:, :rows_c],
                    start=False, stop=True,
                )
            ps_tiles[i] = ps

        def copies(i):
            b, t0c, tch = chunks[i]
            rows_c = tch * U
            row0 = b * (TLOC * U) + t0c * U
            ps = ps_tiles[i]
            for oc in range(O // 128):
                ot = opool.tile([128, CHMAX], F32, tag="ot",
                                name="ot")[:, :rows_c]
                # gpsimd cannot access PSUM; split copies ACT/DVE
                if oc < 2:
                    nc.scalar.activation(ot[:], ps[oc][:],
                                         mybir.ActivationFunctionType.Copy)
                else:
                    nc.vector.tensor_copy(ot[:], ps[oc][:])
                ring = nc.sync if oc % 2 == 0 else nc.scalar
                ring.dma_start(
                    out[oc * 128:(oc + 1) * 128, row0:row0 + rows_c], ot[:])

        build(0)
        for i in range(len(chunks)):
            if i + 1 < len(chunks):
                build(i + 1)
            mms(i)
            copies(i)
    nc.compile()
    return nc


def _chunk128(a):
    # [n*128, w] -> [128, n*w]: partition p holds row k*128+p of chunk k
    n = a.shape[0] // 128
    return np.ascontiguousarray(
        a.reshape(n, 128, a.shape[1]).transpose(1, 0, 2).reshape(128, -1))


def _bf16(a):
    return np.ascontiguousarray(a).astype(ml_dtypes.bfloat16)


def kernel(enc_state, dec_state, W1, b1, W2, b2, _trace=False):
    enc_state = np.ascontiguousarray(enc_state, dtype=np.float32)
    dec_state = np.ascontiguousarray(dec_state, dtype=np.float32)
    W1 = np.asarray(W1, dtype=np.float32)
    b1 = np.asarray(b1, dtype=np.float32)
    W2 = np.asarray(W2, dtype=np.float32)
    b2 = np.asarray(b2, dtype=np.float32)

    if "nc" not in _CACHE:
        _CACHE["nc"] = _build()
    nc = _CACHE["nc"]

    decT = _chunk128(dec_state.reshape(B * U, D).T)
    w1eT = _chunk128(W1[:, :D].T)
    w1dT = _chunk128(W1[:, D:].T)
    w2T = _bf16(_chunk128(W2.T))
    b1r = np.ascontiguousarray(b1.reshape(HK, 128).T)
    inC = _bf16(np.concatenate([decT, w1dT], axis=1))

    in_maps = []
    for c in range(NCORES):
        enc_c = enc_state[:, c * TLOC:(c + 1) * TLOC, :].reshape(PAIRS, D)
        encT = _chunk128(enc_c.T)
        in_maps.append({
            "inS": _bf16(np.concatenate([encT, w1eT], axis=1)),
            "inC": inC, "w2T": w2T, "b1r": b1r,
        })

    res = run_bass_kernel_spmd(nc, in_maps, list(range(NCORES)), trace=_trace)
    out = np.empty((B, T, U, O), dtype=np.float32)
    for c in range(NCORES):
        out[:, c * TLOC:(c + 1) * TLOC] = (
            res.results[c]["out"].T.reshape(B, TLOC, U, O))
    out += b2
    if _trace:
        kernel.last_results = res
    return out


# revision 17
# speedup vs baseline: 1.0586x; 1.0194x over previous
"""RNN-T joint network kernel for 8 Trainium2 NeuronCores.

out[b,t,u,:] = W2 @ tanh(W1e @ enc[b,t] + W1d @ dec[b,u] + b1) + b2

Shapes: B=4, T=200, U=100, D=512, H=1024, O=512 (fp32 in/out).
Sharding: T split 8 ways (25 t's per core); dec + weights replicated.

All matmul inputs are bf16 (rel-err budget 2e-2; measured bf16 error
~3e-3; fp8 measured 3.4e-2 — over budget). bf16/fp32r both stream at
1 cycle/row on the PE, so bf16's win is half the DMA/SBUF traffic and
no fp32r cast instructions at startup.

Input DMA facts measured on HW: descriptor dispatch is ~20ns each, and
the 16 DMA engines are SHARED by both HWDGE rings (~360GB/s total).
So inputs are packed into a few wide tensors (2-5KB lines), ordered by
need: b1 first (it gates the ench bias-copies), then enc-side, then
dec-side, with W1 split k-major in half so phase-1 matmuls start on
partial arrival; W2 last (first needed ~10us later).

Phase 2 emission is software-pipelined by one chunk — build(i+1)
[broadcast-add + tanh], then matmuls(i), then psum copies(i) — so the
in-order DVE/ACT queues always hold ready work ahead of the
PE-dependent psum copies (avoids head-of-line blocking stalls).
Chunks are up to 5 t's (500-row matmul streams; the matmul moving
size is ISA-limited to 512 rows = one PSUM bank).

Engine assignment per chunk: DVE broadcast-adds k0..6 (~110ns per
100-elem line), GpSimd only k7 (it measures ~0.8us per line), ACT does
both tanhs and 2 psum copies, DVE the other 2 (GpSimd cannot read
PSUM).  b2 is added on the host.
"""

from contextlib import ExitStack

import ml_dtypes
import numpy as np

import concourse.bacc as bacc
import concourse.bass as bass
import concourse.mybir as mybir
import concourse.tile as tile
from concourse.bass_utils import run_bass_kernel_spmd

F32 = mybir.dt.float32
BF16 = mybir.dt.bfloat16

B, T, U, D, H, O = 4, 200, 100, 512, 1024, 512
NCORES = 8
TLOC = T // NCORES            # 25 t's per core
PAIRS = B * TLOC              # 100 (b,t) pairs per core
BU = B * U                    # 400
ROWS = PAIRS * U              # 10000 output rows per core
DK = D // 128                 # 4 contraction chunks for phase 1
HK = H // 128                 # 8 h chunks
KA = 7                        # k chunks in the A half (k=0..6, DVE)
KB = HK - KA                  # 1 k chunk in the B half (k=7, GpSimd)
CHMAX = 500                   # max rows per phase-2 chunk

ENC_W = DK * PAIRS            # 400
DEC_W = DK * BU               # 1600
W1H = (HK // 2) * 512         # 2048: k-major half of a W1 side

_CACHE = {}


def _chunks():
    sizes_by_b = [
        [1, 2, 3, 4, 5, 5, 5],
        [5] * 5,
        [5] * 5,
        [5, 5, 5, 5, 4, 1],
    ]
    out = []
    for b, sizes in enumerate(sizes_by_b):
        t0 = 0
        for tch in sizes:
            out.append((b, t0, tch))
            t0 += tch
        assert t0 == TLOC
    return out


def _build():
    nc = bacc.Bacc("TRN2", target_bir_lowering=False, debug=False,
                   num_devices=NCORES)
    # k-major W1 halves: col = (k % 4)*512 + dk*128 + j
    inSa = nc.dram_tensor("inSa", [128, ENC_W + W1H], BF16,
                          kind="ExternalInput")   # encT | w1e k0..3
    inSb = nc.dram_tensor("inSb", [128, W1H], BF16,
                          kind="ExternalInput")   # w1e k4..7
    inCa = nc.dram_tensor("inCa", [128, DEC_W + W1H], BF16,
                          kind="ExternalInput")   # decT | w1d k0..3
    inCb = nc.dram_tensor("inCb", [128, W1H], BF16,
                          kind="ExternalInput")   # w1d k4..7
    w2T = nc.dram_tensor("w2T", [128, HK * O], BF16, kind="ExternalInput")
    b1r = nc.dram_tensor("b1r", [128, HK], F32, kind="ExternalInput")
    out = nc.dram_tensor("out", [O, ROWS], F32, kind="ExternalOutput")

    with tile.TileContext(nc) as tc, ExitStack() as ctx:
        consts = ctx.enter_context(tc.tile_pool(name="consts", bufs=1))
        spoolA = ctx.enter_context(tc.tile_pool(name="spoolA", bufs=3))
        spoolB = ctx.enter_context(tc.tile_pool(name="spoolB", bufs=3))
        opool = ctx.enter_context(tc.tile_pool(name="opool", bufs=8))
        psB = ctx.enter_context(tc.tile_pool(name="psB", bufs=8, space="PSUM"))

        inSa_s = consts.tile([128, ENC_W + W1H], BF16)
        inSb_s = consts.tile([128, W1H], BF16)
        inCa_s = consts.tile([128, DEC_W + W1H], BF16)
        inCb_s = consts.tile([128, W1H], BF16)
        w2_s = consts.tile([128, HK * O], BF16)
        b1_s = consts.tile([128, HK], F32)
        # trigger order by need; b1 first on its ring (it's tiny and
        # gates the ench copies)
        nc.scalar.dma_start(b1_s[:], b1r[:])
        nc.sync.dma_start(inSa_s[:], inSa[:])
        nc.scalar.dma_start(inCa_s[:], inCa[:])
        nc.sync.dma_start(inSb_s[:], inSb[:])
        nc.scalar.dma_start(inCb_s[:], inCb[:])
        nc.sync.dma_start(w2_s[:], w2T[:])
        encT_s = inSa_s[:, :ENC_W]
        decT_s = inCa_s[:, :DEC_W]

        def w1e_blk(k, dk):
            if k < 4:
                return inSa_s[:, ENC_W + k * 512 + dk * 128:
                              ENC_W + k * 512 + (dk + 1) * 128]
            return inSb_s[:, (k - 4) * 512 + dk * 128:
                          (k - 4) * 512 + (dk + 1) * 128]

        def w1d_blk(k, dk):
            if k < 4:
                return inCa_s[:, DEC_W + k * 512 + dk * 128:
                              DEC_W + k * 512 + (dk + 1) * 128]
            return inCb_s[:, (k - 4) * 512 + dk * 128:
                          (k - 4) * 512 + (dk + 1) * 128]

        # ---- phase 1 ----
        ench_t = {"A": consts.tile([128, KA * PAIRS], BF16, name="enchA"),
                  "B": consts.tile([128, KB * PAIRS], BF16, name="enchB")}
        dech_t = {"A": consts.tile([128, KA * BU], BF16, name="dechA"),
                  "B": consts.tile([128, KB * BU], BF16, name="dechB")}

        def halfslot(k):
            return ("A", k) if k < KA else ("B", k - KA)

        def p1_enc(k):
            pe = psB.tile([128, 512], F32, tag="psB", name="pe")[:, :PAIRS]
            for dk in range(DK):
                nc.tensor.matmul(
                    pe[:], lhsT=w1e_blk(k, dk),
                    rhs=encT_s[:, dk * PAIRS:(dk + 1) * PAIRS],
                    start=(dk == 0), stop=(dk == DK - 1),
                )
            hf, kk = halfslot(k)
            nc.scalar.activation(
                ench_t[hf][:, kk * PAIRS:(kk + 1) * PAIRS], pe[:],
                mybir.ActivationFunctionType.Identity, bias=b1_s[:, k:k + 1])

        def p1_dec(k):
            pd = psB.tile([128, 512], F32, tag="psB", name="pd")[:, :BU]
            for dk in range(DK):
                nc.tensor.matmul(
                    pd[:], lhsT=w1d_blk(k, dk),
                    rhs=decT_s[:, dk * BU:(dk + 1) * BU],
                    start=(dk == 0), stop=(dk == DK - 1),
                )
            hf, kk = halfslot(k)
            dst = dech_t[hf][:, kk * BU:(kk + 1) * BU]
            if k % 2 == 0:
                nc.vector.tensor_copy(dst, pd[:])
            else:
                nc.scalar.activation(dst, pd[:],
                                     mybir.ActivationFunctionType.Copy)

        for k in range(4):
            p1_enc(k)
        for k in range(4):
            p1_dec(k)
        for k in range(4, HK):
            p1_enc(k)
        for k in range(4, HK):
            p1_dec(k)

        # ---- phase 2, software-pipelined by one chunk ----
        chunks = _chunks()
        s_tiles = [None] * len(chunks)
        ps_tiles = [None] * len(chunks)

        def build(i):
            b, t0c, tch = chunks[i]
            rows_c = tch * U
            s_t = {"A": spoolA.tile([128, KA * CHMAX], BF16, tag="sA",
                                    name="sA"),
                   "B": spoolB.tile([128, KB * CHMAX], BF16, tag="sB",
                                    name="sB")}
            s_tiles[i] = s_t
            for hf, nk, eng in (("A", KA, nc.vector), ("B", KB, nc.gpsimd)):
                dech_ap = dech_t[hf][:].rearrange(
                    "p (k bu) -> p k bu", k=nk)[:, :, b * U:(b + 1) * U]
                dech_ap = dech_ap.rearrange("p k (a u) -> p k a u", a=1)
                c0 = b * TLOC + t0c
                ench_ap = ench_t[hf][:].rearrange(
                    "p (k c) -> p k c", k=nk)[:, :, c0:c0 + tch]
                ench_ap = ench_ap.rearrange("p k (t a) -> p k t a", a=1)
                bc_d, bc_e = bass.broadcast_tensor_aps(dech_ap, ench_ap)
                outap = s_t[hf][:, :nk * rows_c].rearrange(
                    "p (k t u) -> p k t u", k=nk, t=tch)
                eng.tensor_tensor(outap, bc_d, bc_e, mybir.AluOpType.add)
                nc.scalar.activation(s_t[hf][:, :nk * rows_c],
                                     s_t[hf][:, :nk * rows_c],
                                     mybir.ActivationFunctionType.Tanh)

        def mms(i):
            b, t0c, tch = chunks[i]
            rows_c = tch * U
            s_t = s_tiles[i]
            ps = []
            for oc in range(O // 128):
                p = psB.tile([128, 512], F32, tag="psB",
                             name="p")[:, :rows_c]
                ps.append(p)
                for k in range(KA):
                    nc.tensor.matmul(
                        p[:],
                        lhsT=w2_s[:, k * O + oc * 128: k * O + (oc + 1) * 128],
                        rhs=s_t["A"][:, k * rows_c:(k + 1) * rows_c],
                        start=(k == 0), stop=False,
                    )
                k = KA
                nc.tensor.matmul(
                    p[:],
                    lhsT=w2_s[:, k * O + oc * 128: k * O + (oc + 1) * 128],
                    rhs=s_t["B"][:, :rows_c],
                    start=False, stop=True,
                )
            ps_tiles[i] = ps

        def copies(i):
            b, t0c, tch = chunks[i]
            rows_c = tch * U
            row0 = b * (TLOC * U) + t0c * U
            ps = ps_tiles[i]
            for oc in range(O // 128):
                ot = opool.tile([128, CHMAX], F32, tag="ot",
                                name="ot")[:, :rows_c]
                # gpsimd cannot access PSUM; split copies ACT/DVE
                if oc < 2:
                    nc.scalar.activation(ot[:], ps[oc][:],
                                         mybir.ActivationFunctionType.Copy)
                else:
                    nc.vector.tensor_copy(ot[:], ps[oc][:])
                ring = nc.sync if oc % 2 == 0 else nc.scalar
                ring.dma_start(
                    out[oc * 128:(oc + 1) * 128, row0:row0 + rows_c], ot[:])

        build(0)
        for i in range(len(chunks)):
            if i + 1 < len(chunks):
                build(i + 1)
            mms(i)
            copies(i)
    nc.compile()
    return nc


def _chunk128(a):
    # [n*128, w] -> [128, n*w]: partition p holds row k*128+p of chunk k
    n = a.shape[0] // 128
    return np.ascontiguousarray(
        a.reshape(n, 128, a.shape[1]).transpose(1, 0, 2).reshape(128, -1))


def _bf16(a):
    return np.ascontiguousarray(a).astype(ml_dtypes.bfloat16)


def _kmajor(w1T):
    # [128, dk-major (DK x H)] -> [128, k-major (HK x DK x 128)]
    return np.ascontiguousarray(
        w1T.reshape(128, DK, HK, 128).transpose(0, 2, 1, 3).reshape(128, -1))


def kernel(enc_state, dec_state, W1, b1, W2, b2, _trace=False):
    enc_state = np.ascontiguousarray(enc_state, dtype=np.float32)
    dec_state = np.ascontiguousarray(dec_state, dtype=np.float32)
    W1 = np.asarray(W1, dtype=np.float32)
    b1 = np.asarray(b1, dtype=np.float32)
    W2 = np.asarray(W2, dtype=np.float32)
    b2 = np.asarray(b2, dtype=np.float32)

    if "nc" not in _CACHE:
        _CACHE["nc"] = _build()
    nc = _CACHE["nc"]

    decT = _chunk128(dec_state.reshape(B * U, D).T)
    w1e_km = _kmajor(_chunk128(W1[:, :D].T))
    w1d_km = _kmajor(_chunk128(W1[:, D:].T))
    w2T = _bf16(_chunk128(W2.T))
    b1r = np.ascontiguousarray(b1.reshape(HK, 128).T)
    inCa = _bf16(np.concatenate([decT, w1d_km[:, :W1H]], axis=1))
    inCb = _bf16(w1d_km[:, W1H:])
    inSb = _bf16(w1e_km[:, W1H:])

    in_maps = []
    for c in range(NCORES):
        enc_c = enc_state[:, c * TLOC:(c + 1) * TLOC, :].reshape(PAIRS, D)
        encT = _chunk128(enc_c.T)
        in_maps.append({
            "inSa": _bf16(np.concatenate([encT, w1e_km[:, :W1H]], axis=1)),
            "inSb": inSb, "inCa": inCa, "inCb": inCb,
            "w2T": w2T, "b1r": b1r,
        })

    res = run_bass_kernel_spmd(nc, in_maps, list(range(NCORES)), trace=_trace)
    out = np.empty((B, T, U, O), dtype=np.float32)
    for c in range(NCORES):
        out[:, c * TLOC:(c + 1) * TLOC] = (
            res.results[c]["out"].T.reshape(B, TLOC, U, O))
    out += b2
    if _trace:
        kernel.last_results = res
    return out


# revision 21
# speedup vs baseline: 1.0692x; 1.0100x over previous
"""RNN-T joint network kernel for 8 Trainium2 NeuronCores.

out[b,t,u,:] = W2 @ tanh(W1e @ enc[b,t] + W1d @ dec[b,u] + b1) + b2

Shapes: B=4, T=200, U=100, D=512, H=1024, O=512 (fp32 in/out).
Sharding: T split 8 ways (25 t's per core); dec + weights replicated.

All matmul inputs are bf16 (rel-err budget 2e-2; measured bf16 error
~3e-3; fp8 measured 3.4e-2 — over budget). bf16/fp32r both stream at
1 cycle/row on the PE, so bf16's win is half the DMA/SBUF traffic and
no fp32r cast instructions at startup.

Input DMA facts measured on HW: descriptor dispatch is ~20ns each, and
the 16 DMA engines are SHARED by both HWDGE rings (~360GB/s total).
So inputs are packed into a few wide tensors (2-5KB lines), ordered by
need: b1 first (it gates the ench bias-copies), then enc-side, then
dec-side, with W1 split k-major in half so phase-1 matmuls start on
partial arrival; W2 last (first needed ~10us later).

Phase 2 emission is software-pipelined by one chunk — build(i+1)
[broadcast-add + tanh], then matmuls(i), then psum copies(i) — so the
in-order DVE/ACT queues always hold ready work ahead of the
PE-dependent psum copies (avoids head-of-line blocking stalls).
Chunks are up to 5 t's (500-row matmul streams; the matmul moving
size is ISA-limited to 512 rows = one PSUM bank).

Engine assignment per chunk: DVE broadcast-adds k0..6 (~110ns per
100-elem line), GpSimd only k7 (it measures ~0.8us per line), ACT does
both tanhs and 2 psum copies, DVE the other 2 (GpSimd cannot read
PSUM).  b2 is added on the host.
"""

from contextlib import ExitStack

import ml_dtypes
import numpy as np

import concourse.bacc as bacc
import concourse.bass as bass
import concourse.mybir as mybir
import concourse.tile as tile
from concourse.bass_utils import run_bass_kernel_spmd

F32 = mybir.dt.float32
BF16 = mybir.dt.bfloat16

B, T, U, D, H, O = 4, 200, 100, 512, 1024, 512
NCORES = 8
TLOC = T // NCORES            # 25 t's per core
PAIRS = B * TLOC              # 100 (b,t) pairs per core
BU = B * U                    # 400
ROWS = PAIRS * U              # 10000 output rows per core
DK = D // 128                 # 4 contraction chunks for phase 1
HK = H // 128                 # 8 h chunks
KA = 7                        # k chunks in the A half (k=0..6, DVE)
KB = HK - KA                  # 1 k chunk in the B half (k=7, GpSimd)
CHMAX = 500                   # max rows per phase-2 chunk

ENC_W = DK * PAIRS            # 400
DEC_W = DK * BU               # 1600
W1H = (HK // 2) * 512         # 2048: k-major half of a W1 side

_CACHE = {}


def _chunks():
    sizes_by_b = [
        [1, 2, 3, 4, 5, 5, 5],
        [5] * 5,
        [5] * 5,
        [5, 5, 5, 5, 4, 1],
    ]
    out = []
    for b, sizes in enumerate(sizes_by_b):
        t0 = 0
        for tch in sizes:
            out.append((b, t0, tch))
            t0 += tch
        assert t0 == TLOC
    return out


def _build():
    nc = bacc.Bacc("TRN2", target_bir_lowering=False, debug=False,
                   num_devices=NCORES)
    # k-major W1 halves: col = (k % 4)*512 + dk*128 + j
    inSa = nc.dram_tensor("inSa", [128, ENC_W + W1H], BF16,
                          kind="ExternalInput")   # encT | w1e k0..3
    inSb = nc.dram_tensor("inSb", [128, W1H], BF16,
                          kind="ExternalInput")   # w1e k4..7
    inCa = nc.dram_tensor("inCa", [128, DEC_W + W1H], BF16,
                          kind="ExternalInput")   # decT | w1d k0..3
    inCb = nc.dram_tensor("inCb", [128, W1H], BF16,
                          kind="ExternalInput")   # w1d k4..7
    w2T = nc.dram_tensor("w2T", [128, HK * O], BF16, kind="ExternalInput")
    b1r = nc.dram_tensor("b1r", [128, HK], F32, kind="ExternalInput")
    out = nc.dram_tensor("out", [O, ROWS], BF16, kind="ExternalOutput")

    with tile.TileContext(nc) as tc, ExitStack() as ctx:
        consts = ctx.enter_context(tc.tile_pool(name="consts", bufs=1))
        spoolA = ctx.enter_context(tc.tile_pool(name="spoolA", bufs=3))
        spoolB = ctx.enter_context(tc.tile_pool(name="spoolB", bufs=3))
        opool = ctx.enter_context(tc.tile_pool(name="opool", bufs=8))
        psB = ctx.enter_context(tc.tile_pool(name="psB", bufs=8, space="PSUM"))

        inSa_s = consts.tile([128, ENC_W + W1H], BF16)
        inSb_s = consts.tile([128, W1H], BF16)
        inCa_s = consts.tile([128, DEC_W + W1H], BF16)
        inCb_s = consts.tile([128, W1H], BF16)
        w2_s = consts.tile([128, HK * O], BF16)
        b1_s = consts.tile([128, HK], F32)
        # Within a ring, descriptors dispatch FIFO in trigger order and
        # the 16 DMA engines are shared across rings — so the late-needed
        # W2 (first used ~24us in) goes to the BACK of the scalar ring's
        # FIFO, and b1 (which gates the ench copies) goes first.
        nc.scalar.dma_start(b1_s[:], b1r[:])
        nc.sync.dma_start(inSa_s[:], inSa[:])
        nc.scalar.dma_start(inCa_s[:], inCa[:])
        nc.sync.dma_start(inSb_s[:], inSb[:])
        nc.scalar.dma_start(inCb_s[:], inCb[:])
        nc.scalar.dma_start(w2_s[:], w2T[:])
        encT_s = inSa_s[:, :ENC_W]
        decT_s = inCa_s[:, :DEC_W]

        def w1e_blk(k, dk):
            if k < 4:
                return inSa_s[:, ENC_W + k * 512 + dk * 128:
                              ENC_W + k * 512 + (dk + 1) * 128]
            return inSb_s[:, (k - 4) * 512 + dk * 128:
                          (k - 4) * 512 + (dk + 1) * 128]

        def w1d_blk(k, dk):
            if k < 4:
                return inCa_s[:, DEC_W + k * 512 + dk * 128:
                              DEC_W + k * 512 + (dk + 1) * 128]
            return inCb_s[:, (k - 4) * 512 + dk * 128:
                          (k - 4) * 512 + (dk + 1) * 128]

        # ---- phase 1 ----
        ench_t = {"A": consts.tile([128, KA * PAIRS], BF16, name="enchA"),
                  "B": consts.tile([128, KB * PAIRS], BF16, name="enchB")}
        dech_t = {"A": consts.tile([128, KA * BU], BF16, name="dechA"),
                  "B": consts.tile([128, KB * BU], BF16, name="dechB")}

        def halfslot(k):
            return ("A", k) if k < KA else ("B", k - KA)

        def p1_enc(k):
            pe = psB.tile([128, 512], F32, tag="psB", name="pe")[:, :PAIRS]
            for dk in range(DK):
                nc.tensor.matmul(
                    pe[:], lhsT=w1e_blk(k, dk),
                    rhs=encT_s[:, dk * PAIRS:(dk + 1) * PAIRS],
                    start=(dk == 0), stop=(dk == DK - 1),
                )
            hf, kk = halfslot(k)
            nc.scalar.activation(
                ench_t[hf][:, kk * PAIRS:(kk + 1) * PAIRS], pe[:],
                mybir.ActivationFunctionType.Identity, bias=b1_s[:, k:k + 1])

        def p1_dec(k):
            pd = psB.tile([128, 512], F32, tag="psB", name="pd")[:, :BU]
            for dk in range(DK):
                nc.tensor.matmul(
                    pd[:], lhsT=w1d_blk(k, dk),
                    rhs=decT_s[:, dk * BU:(dk + 1) * BU],
                    start=(dk == 0), stop=(dk == DK - 1),
                )
            hf, kk = halfslot(k)
            dst = dech_t[hf][:, kk * BU:(kk + 1) * BU]
            if k % 2 == 0:
                nc.vector.tensor_copy(dst, pd[:])
            else:
                nc.scalar.activation(dst, pd[:],
                                     mybir.ActivationFunctionType.Copy)

        for k in range(4):
            p1_enc(k)
        for k in range(4):
            p1_dec(k)
        for k in range(4, HK):
            p1_enc(k)
        for k in range(4, HK):
            p1_dec(k)

        # ---- phase 2, software-pipelined by one chunk ----
        chunks = _chunks()
        s_tiles = [None] * len(chunks)
        ps_tiles = [None] * len(chunks)

        def build(i):
            b, t0c, tch = chunks[i]
            rows_c = tch * U
            s_t = {"A": spoolA.tile([128, KA * CHMAX], BF16, tag="sA",
                                    name="sA"),
                   "B": spoolB.tile([128, KB * CHMAX], BF16, tag="sB",
                                    name="sB")}
            s_tiles[i] = s_t
            for hf, nk, eng in (("A", KA, nc.vector), ("B", KB, nc.gpsimd)):
                dech_ap = dech_t[hf][:].rearrange(
                    "p (k bu) -> p k bu", k=nk)[:, :, b * U:(b + 1) * U]
                dech_ap = dech_ap.rearrange("p k (a u) -> p k a u", a=1)
                c0 = b * TLOC + t0c
                ench_ap = ench_t[hf][:].rearrange(
                    "p (k c) -> p k c", k=nk)[:, :, c0:c0 + tch]
                ench_ap = ench_ap.rearrange("p k (t a) -> p k t a", a=1)
                bc_d, bc_e = bass.broadcast_tensor_aps(dech_ap, ench_ap)
                outap = s_t[hf][:, :nk * rows_c].rearrange(
                    "p (k t u) -> p k t u", k=nk, t=tch)
                eng.tensor_tensor(outap, bc_d, bc_e, mybir.AluOpType.add)
                nc.scalar.activation(s_t[hf][:, :nk * rows_c],
                                     s_t[hf][:, :nk * rows_c],
                                     mybir.ActivationFunctionType.Tanh)

        def mms(i):
            b, t0c, tch = chunks[i]
            rows_c = tch * U
            s_t = s_tiles[i]
            ps = []
            for oc in range(O // 128):
                p = psB.tile([128, 512], F32, tag="psB",
                             name="p")[:, :rows_c]
                ps.append(p)
                for k in range(KA):
                    nc.tensor.matmul(
                        p[:],
                        lhsT=w2_s[:, k * O + oc * 128: k * O + (oc + 1) * 128],
                        rhs=s_t["A"][:, k * rows_c:(k + 1) * rows_c],
                        start=(k == 0), stop=False,
                    )
                k = KA
                nc.tensor.matmul(
                    p[:],
                    lhsT=w2_s[:, k * O + oc * 128: k * O + (oc + 1) * 128],
                    rhs=s_t["B"][:, :rows_c],
                    start=False, stop=True,
                )
            ps_tiles[i] = ps

        def copies(i):
            b, t0c, tch = chunks[i]
            rows_c = tch * U
            row0 = b * (TLOC * U) + t0c * U
            ps = ps_tiles[i]
            for oc in range(O // 128):
                ot = opool.tile([128, CHMAX], BF16, tag="ot",
                                name="ot")[:, :rows_c]
                # gpsimd cannot access PSUM; split copies ACT/DVE
                if oc < 2:
                    nc.scalar.activation(ot[:], ps[oc][:],
                                         mybir.ActivationFunctionType.Copy)
                else:
                    nc.vector.tensor_copy(ot[:], ps[oc][:])
                ring = nc.sync if oc % 2 == 0 else nc.scalar
                ring.dma_start(
                    out[oc * 128:(oc + 1) * 128, row0:row0 + rows_c], ot[:])

        build(0)
        for i in range(len(chunks)):
            if i + 1 < len(chunks):
                build(i + 1)
            mms(i)
            copies(i)
    nc.compile()
    return nc


def _chunk128(a):
    # [n*128, w] -> [128, n*w]: partition p holds row k*128+p of chunk k
    n = a.shape[0] // 128
    return np.ascontiguousarray(
        a.reshape(n, 128, a.shape[1]).transpose(1, 0, 2).reshape(128, -1))


def _bf16(a):
    return np.ascontiguousarray(a).astype(ml_dtypes.bfloat16)


def _kmajor(w1T):
    # [128, dk-major (DK x H)] -> [128, k-major (HK x DK x 128)]
    return np.ascontiguousarray(
        w1T.reshape(128, DK, HK, 128).transpose(0, 2, 1, 3).reshape(128, -1))


def kernel(enc_state, dec_state, W1, b1, W2, b2, _trace=False):
    enc_state = np.ascontiguousarray(enc_state, dtype=np.float32)
    dec_state = np.ascontiguousarray(dec_state, dtype=np.float32)
    W1 = np.asarray(W1, dtype=np.float32)
    b1 = np.asarray(b1, dtype=np.float32)
    W2 = np.asarray(W2, dtype=np.float32)
    b2 = np.asarray(b2, dtype=np.float32)

    if "nc" not in _CACHE:
        _CACHE["nc"] = _build()
    nc = _CACHE["nc"]

    decT = _chunk128(dec_state.reshape(B * U, D).T)
    w1e_km = _kmajor(_chunk128(W1[:, :D].T))
    w1d_km = _kmajor(_chunk128(W1[:, D:].T))
    w2T = _bf16(_chunk128(W2.T))
    b1r = np.ascontiguousarray(b1.reshape(HK, 128).T)
    inCa = _bf16(np.concatenate([decT, w1d_km[:, :W1H]], axis=1))
    inCb = _bf16(w1d_km[:, W1H:])
    inSb = _bf16(w1e_km[:, W1H:])

    in_maps = []
    for c in range(NCORES):
        enc_c = enc_state[:, c * TLOC:(c + 1) * TLOC, :].reshape(PAIRS, D)
        encT = _chunk128(enc_c.T)
        in_maps.append({
            "inSa": _bf16(np.concatenate([encT, w1e_km[:, :W1H]], axis=1)),
            "inSb": inSb, "inCa": inCa, "inCb": inCb,
            "w2T": w2T, "b1r": b1r,
        })

    res = run_bass_kernel_spmd(nc, in_maps, list(range(NCORES)), trace=_trace)
    out = np.empty((B, T, U, O), dtype=np.float32)
    for c in range(NCORES):
        out[:, c * TLOC:(c + 1) * TLOC] = (
            res.results[c]["out"].astype(np.float32).T.reshape(
                B, TLOC, U, O))
    out += b2
    if _trace:
        kernel.last_results = res
    return out


# revision 22
# speedup vs baseline: 1.0968x; 1.0259x over previous
"""RNN-T joint network kernel for 8 Trainium2 NeuronCores.

out[b,t,u,:] = W2 @ tanh(W1e @ enc[b,t] + W1d @ dec[b,u] + b1) + b2

Shapes: B=4, T=200, U=100, D=512, H=1024, O=512 (fp32 in/out).
Sharding: T split 8 ways (25 t's per core); dec + weights replicated.

All matmul inputs are bf16 (rel-err budget 2e-2; measured bf16 error
~3.8e-3; fp8 measured 3.4e-2 — over budget). bf16/fp32r both stream
at 1 cycle/row on the PE, so bf16's win is half the DMA/SBUF traffic
and no fp32r cast instructions at startup.

Input DMA facts measured on HW: the 16 DMA engines are shared by both
HWDGE rings (~370 B/ns aggregate) and each ring dispatches its queue
FIFO. Inputs are packed into wide tensors (2-11KB lines) and ordered
by need: b1 and the b=0 slice of the dec side first, W2 (first needed
~24us in) at the back of the scalar ring's FIFO.

Phase 1 is split by batch: only dech[b=0] is computed up front (16
fast 100-col matmuls); the 300-col dec matmuls for b=1..3 are emitted
as PE filler between the first chunk groups, exactly where the b0
build chain (add -> tanh) would otherwise stall the PE.

Phase 2 emission is software-pipelined by one chunk — build(i+1)
[broadcast-add + tanh], then matmuls(i), then psum copies(i) — so the
in-order DVE/ACT queues always hold ready work ahead of the
PE-dependent psum copies (avoids head-of-line blocking).  Chunks are
up to 5 t's (500-row streams; matmul moving size is ISA-limited to
512 = one PSUM bank).

Engine assignment per chunk: DVE broadcast-adds k0..6 (~110ns per
100-elem line), GpSimd only k7 (it measures ~0.8us per line), ACT does
both tanhs and 2 psum copies, DVE the other 2 (GpSimd cannot read
PSUM). Output is written bf16 (halves out-DMA); b2 and the fp32
upcast happen on the host.
"""

from contextlib import ExitStack

import ml_dtypes
import numpy as np

import concourse.bacc as bacc
import concourse.bass as bass
import concourse.mybir as mybir
import concourse.tile as tile
from concourse.bass_utils import run_bass_kernel_spmd

F32 = mybir.dt.float32
BF16 = mybir.dt.bfloat16

B, T, U, D, H, O = 4, 200, 100, 512, 1024, 512
NCORES = 8
TLOC = T // NCORES            # 25 t's per core
PAIRS = B * TLOC              # 100 (b,t) pairs per core
BU = B * U                    # 400
ROWS = PAIRS * U              # 10000 output rows per core
DK = D // 128                 # 4 contraction chunks for phase 1
HK = H // 128                 # 8 h chunks
KA = 7                        # k chunks in the A half (k=0..6, DVE)
KB = HK - KA                  # 1 k chunk in the B half (k=7, GpSimd)
CHMAX = 500                   # max rows per phase-2 chunk
U3 = 3 * U                    # 300 dec cols for b=1..3

ENC_W = DK * PAIRS            # 400
DEC0_W = DK * U               # 400: b=0 slice of decT
DEC3_W = DK * U3              # 1200: b=1..3 slice
W1H = (HK // 2) * 512         # 2048: k-major half of a W1 side

_CACHE = {}


def _chunks():
    sizes_by_b = [
        [1, 2, 3, 4, 5, 5, 5],
        [5] * 5,
        [5] * 5,
        [5, 5, 5, 5, 4, 1],
    ]
    out = []
    for b, sizes in enumerate(sizes_by_b):
        t0 = 0
        for tch in sizes:
            out.append((b, t0, tch))
            t0 += tch
        assert t0 == TLOC
    return out


def _build():
    nc = bacc.Bacc("TRN2", target_bir_lowering=False, debug=False,
                   num_devices=NCORES)
    # k-major W1 halves: col = (k % 4)*512 + dk*128 + j
    inSa = nc.dram_tensor("inSa", [128, ENC_W + W1H], BF16,
                          kind="ExternalInput")   # encT | w1e k0..3
    inSb = nc.dram_tensor("inSb", [128, W1H], BF16,
                          kind="ExternalInput")   # w1e k4..7
    inCa = nc.dram_tensor("inCa", [128, DEC0_W + W1H], BF16,
                          kind="ExternalInput")   # decT b0 | w1d k0..3
    inCb = nc.dram_tensor("inCb", [128, W1H], BF16,
                          kind="ExternalInput")   # w1d k4..7
    dec3 = nc.dram_tensor("dec3", [128, DEC3_W], BF16,
                          kind="ExternalInput")   # decT b1..3
    w2T = nc.dram_tensor("w2T", [128, HK * O], BF16, kind="ExternalInput")
    b1r = nc.dram_tensor("b1r", [128, HK], F32, kind="ExternalInput")
    out = nc.dram_tensor("out", [O, ROWS], BF16, kind="ExternalOutput")

    with tile.TileContext(nc) as tc, ExitStack() as ctx:
        consts = ctx.enter_context(tc.tile_pool(name="consts", bufs=1))
        spoolA = ctx.enter_context(tc.tile_pool(name="spoolA", bufs=3))
        spoolB = ctx.enter_context(tc.tile_pool(name="spoolB", bufs=3))
        opool = ctx.enter_context(tc.tile_pool(name="opool", bufs=8))
        psB = ctx.enter_context(tc.tile_pool(name="psB", bufs=8, space="PSUM"))

        inSa_s = consts.tile([128, ENC_W + W1H], BF16)
        inSb_s = consts.tile([128, W1H], BF16)
        inCa_s = consts.tile([128, DEC0_W + W1H], BF16)
        inCb_s = consts.tile([128, W1H], BF16)
        dec3_s = consts.tile([128, DEC3_W], BF16)
        w2_s = consts.tile([128, HK * O], BF16)
        b1_s = consts.tile([128, HK], F32)
        nc.scalar.dma_start(b1_s[:], b1r[:])
        nc.sync.dma_start(inSa_s[:], inSa[:])
        nc.scalar.dma_start(inCa_s[:], inCa[:])
        nc.sync.dma_start(inSb_s[:], inSb[:])
        nc.scalar.dma_start(inCb_s[:], inCb[:])
        nc.scalar.dma_start(dec3_s[:], dec3[:])
        nc.scalar.dma_start(w2_s[:], w2T[:])
        encT_s = inSa_s[:, :ENC_W]
        dec0_s = inCa_s[:, :DEC0_W]

        def w1e_blk(k, dk):
            if k < 4:
                return inSa_s[:, ENC_W + k * 512 + dk * 128:
                              ENC_W + k * 512 + (dk + 1) * 128]
            return inSb_s[:, (k - 4) * 512 + dk * 128:
                          (k - 4) * 512 + (dk + 1) * 128]

        def w1d_blk(k, dk):
            if k < 4:
                return inCa_s[:, DEC0_W + k * 512 + dk * 128:
                              DEC0_W + k * 512 + (dk + 1) * 128]
            return inCb_s[:, (k - 4) * 512 + dk * 128:
                          (k - 4) * 512 + (dk + 1) * 128]

        # ---- phase 1 tiles ----
        ench_t = {"A": consts.tile([128, KA * PAIRS], BF16, name="enchA"),
                  "B": consts.tile([128, KB * PAIRS], BF16, name="enchB")}
        dech0 = {"A": consts.tile([128, KA * U], BF16, name="dech0A"),
                 "B": consts.tile([128, KB * U], BF16, name="dech0B")}
        dech3 = {"A": consts.tile([128, KA * U3], BF16, name="dech3A"),
                 "B": consts.tile([128, KB * U3], BF16, name="dech3B")}

        def halfslot(k):
            return ("A", k) if k < KA else ("B", k - KA)

        def p1_enc(k):
            pe = psB.tile([128, 512], F32, tag="psB", name="pe")[:, :PAIRS]
            for dk in range(DK):
                nc.tensor.matmul(
                    pe[:], lhsT=w1e_blk(k, dk),
                    rhs=encT_s[:, dk * PAIRS:(dk + 1) * PAIRS],
                    start=(dk == 0), stop=(dk == DK - 1),
                )
            hf, kk = halfslot(k)
            nc.scalar.activation(
                ench_t[hf][:, kk * PAIRS:(kk + 1) * PAIRS], pe[:],
                mybir.ActivationFunctionType.Identity, bias=b1_s[:, k:k + 1])

        def p1_dec0(k):
            pd = psB.tile([128, 512], F32, tag="psB", name="pd")[:, :U]
            for dk in range(DK):
                nc.tensor.matmul(
                    pd[:], lhsT=w1d_blk(k, dk),
                    rhs=dec0_s[:, dk * U:(dk + 1) * U],
                    start=(dk == 0), stop=(dk == DK - 1),
                )
            hf, kk = halfslot(k)
            dst = dech0[hf][:, kk * U:(kk + 1) * U]
            if k % 2 == 0:
                nc.vector.tensor_copy(dst, pd[:])
            else:
                nc.scalar.activation(dst, pd[:],
                                     mybir.ActivationFunctionType.Copy)

        def p1_dec3(k):
            pd = psB.tile([128, 512], F32, tag="psB", name="pd3")[:, :U3]
            for dk in range(DK):
                nc.tensor.matmul(
                    pd[:], lhsT=w1d_blk(k, dk),
                    rhs=dec3_s[:, dk * U3:(dk + 1) * U3],
                    start=(dk == 0), stop=(dk == DK - 1),
                )
            hf, kk = halfslot(k)
            dst = dech3[hf][:, kk * U3:(kk + 1) * U3]
            if k % 2 == 0:
                nc.vector.tensor_copy(dst, pd[:])
            else:
                nc.scalar.activation(dst, pd[:],
                                     mybir.ActivationFunctionType.Copy)

        for k in range(4):
            p1_enc(k)
        for k in range(4):
            p1_dec0(k)
        for k in range(4, HK):
            p1_enc(k)
        for k in range(4, HK):
            p1_dec0(k)

        # ---- phase 2, software-pipelined by one chunk ----
        chunks = _chunks()
        s_tiles = [None] * len(chunks)
        ps_tiles = [None] * len(chunks)

        def build(i):
            b, t0c, tch = chunks[i]
            rows_c = tch * U
            s_t = {"A": spoolA.tile([128, KA * CHMAX], BF16, tag="sA",
                                    name="sA"),
                   "B": spoolB.tile([128, KB * CHMAX], BF16, tag="sB",
                                    name="sB")}
            s_tiles[i] = s_t
            for hf, nk, eng in (("A", KA, nc.vector), ("B", KB, nc.gpsimd)):
                if b == 0:
                    dech_ap = dech0[hf][:].rearrange(
                        "p (k u) -> p k u", k=nk)
                else:
                    dech_ap = dech3[hf][:].rearrange(
                        "p (k bu) -> p k bu", k=nk)[
                            :, :, (b - 1) * U:b * U]
                dech_ap = dech_ap.rearrange("p k (a u) -> p k a u", a=1)
                c0 = b * TLOC + t0c
                ench_ap = ench_t[hf][:].rearrange(
                    "p (k c) -> p k c", k=nk)[:, :, c0:c0 + tch]
                ench_ap = ench_ap.rearrange("p k (t a) -> p k t a", a=1)
                bc_d, bc_e = bass.broadcast_tensor_aps(dech_ap, ench_ap)
                outap = s_t[hf][:, :nk * rows_c].rearrange(
                    "p (k t u) -> p k t u", k=nk, t=tch)
                eng.tensor_tensor(outap, bc_d, bc_e, mybir.AluOpType.add)
                nc.scalar.activation(s_t[hf][:, :nk * rows_c],
                                     s_t[hf][:, :nk * rows_c],
                                     mybir.ActivationFunctionType.Tanh)

        def mms(i):
            b, t0c, tch = chunks[i]
            rows_c = tch * U
            s_t = s_tiles[i]
            ps = []
            for oc in range(O // 128):
                p = psB.tile([128, 512], F32, tag="psB",
                             name="p")[:, :rows_c]
                ps.append(p)
                for k in range(KA):
                    nc.tensor.matmul(
                        p[:],
                        lhsT=w2_s[:, k * O + oc * 128: k * O + (oc + 1) * 128],
                        rhs=s_t["A"][:, k * rows_c:(k + 1) * rows_c],
                        start=(k == 0), stop=False,
                    )
                k = KA
                nc.tensor.matmul(
                    p[:],
                    lhsT=w2_s[:, k * O + oc * 128: k * O + (oc + 1) * 128],
                    rhs=s_t["B"][:, :rows_c],
                    start=False, stop=True,
                )
            ps_tiles[i] = ps

        def copies(i):
            b, t0c, tch = chunks[i]
            rows_c = tch * U
            row0 = b * (TLOC * U) + t0c * U
            ps = ps_tiles[i]
            for oc in range(O // 128):
                ot = opool.tile([128, CHMAX], BF16, tag="ot",
                                name="ot")[:, :rows_c]
                # gpsimd cannot access PSUM; split copies ACT/DVE
                if oc < 2:
                    nc.scalar.activation(ot[:], ps[oc][:],
                                         mybir.ActivationFunctionType.Copy)
                else:
                    nc.vector.tensor_copy(ot[:], ps[oc][:])
                ring = nc.sync if oc % 2 == 0 else nc.scalar
                ring.dma_start(
                    out[oc * 128:(oc + 1) * 128, row0:row0 + rows_c], ot[:])

        # prologue: the dec matmuls for b=1..3 are PE filler while the
        # b0 build chain (DVE add -> ACT tanh) catches up
        build(0)
        build(1)
        mms(0)
        copies(0)
        build(2)
        for k in range(4):
            p1_dec3(k)
        mms(1)
        copies(1)
        build(3)
        for k in range(4, HK):
            p1_dec3(k)
        mms(2)
        copies(2)
        for i in range(3, len(chunks)):
            if i + 1 < len(chunks):
                build(i + 1)
            mms(i)
            copies(i)
    nc.compile()
    return nc


def _chunk128(a):
    # [n*128, w] -> [128, n*w]: partition p holds row k*128+p of chunk k
    n = a.shape[0] // 128
    return np.ascontiguousarray(
        a.reshape(n, 128, a.shape[1]).transpose(1, 0, 2).reshape(128, -1))


def _bf16(a):
    return np.ascontiguousarray(a).astype(ml_dtypes.bfloat16)


def _kmajor(w1T):
    # [128, dk-major (DK x H)] -> [128, k-major (HK x DK x 128)]
    return np.ascontiguousarray(
        w1T.reshape(128, DK, HK, 128).transpose(0, 2, 1, 3).reshape(128, -1))


def kernel(enc_state, dec_state, W1, b1, W2, b2, _trace=False):
    enc_state = np.ascontiguousarray(enc_state, dtype=np.float32)
    dec_state = np.ascontiguousarray(dec_state, dtype=np.float32)
    W1 = np.asarray(W1, dtype=np.float32)
    b1 = np.asarray(b1, dtype=np.float32)
    W2 = np.asarray(W2, dtype=np.float32)
    b2 = np.asarray(b2, dtype=np.float32)

    if "nc" not in _CACHE:
        _CACHE["nc"] = _build()
    nc = _CACHE["nc"]

    decT = _chunk128(dec_state.reshape(B * U, D).T)     # [128, DK*BU]
    dec0 = np.concatenate(
        [decT[:, dk * BU: dk * BU + U] for dk in range(DK)], axis=1)
    dec3 = np.concatenate(
        [decT[:, dk * BU + U:(dk + 1) * BU] for dk in range(DK)], axis=1)
    w1e_km = _kmajor(_chunk128(W1[:, :D].T))
    w1d_km = _kmajor(_chunk128(W1[:, D:].T))
    w2T = _bf16(_chunk128(W2.T))
    b1r = np.ascontiguousarray(b1.reshape(HK, 128).T)
    inCa = _bf16(np.concatenate([dec0, w1d_km[:, :W1H]], axis=1))
    inCb = _bf16(w1d_km[:, W1H:])
    inSb = _bf16(w1e_km[:, W1H:])
    dec3 = _bf16(dec3)

    in_maps = []
    for c in range(NCORES):
        enc_c = enc_state[:, c * TLOC:(c + 1) * TLOC, :].reshape(PAIRS, D)
        encT = _chunk128(enc_c.T)
        in_maps.append({
            "inSa": _bf16(np.concatenate([encT, w1e_km[:, :W1H]], axis=1)),
            "inSb": inSb, "inCa": inCa, "inCb": inCb, "dec3": dec3,
            "w2T": w2T, "b1r": b1r,
        })

    res = run_bass_kernel_spmd(nc, in_maps, list(range(NCORES)), trace=_trace)
    out = np.empty((B, T, U, O), dtype=np.float32)
    for c in range(NCORES):
        out[:, c * TLOC:(c + 1) * TLOC] = (
            res.results[c]["out"].astype(np.float32).T.reshape(
                B, TLOC, U, O))
    out += b2
    if _trace:
        kernel.last_results = res
    return out


# revision 26
# speedup vs baseline: 1.1090x; 1.0111x over previous
"""RNN-T joint network kernel for 8 Trainium2 NeuronCores.

out[b,t,u,:] = W2 @ tanh(W1e @ enc[b,t] + W1d @ dec[b,u] + b1) + b2

Shapes: B=4, T=200, U=100, D=512, H=1024, O=512 (fp32 in/out).
Sharding: T split 8 ways (25 t's per core); dec + weights replicated.

All matmul inputs are bf16 (rel-err budget 2e-2; measured bf16 error
~3.8e-3; fp8 measured 3.4e-2 — over budget). bf16/fp32r both stream
at 1 cycle/row on the PE, so bf16's win is half the DMA/SBUF traffic
and no fp32r cast instructions at startup.

Input DMA facts measured on HW: the 16 DMA engines are shared by both
HWDGE rings (~370 B/ns aggregate) and each ring dispatches its queue
FIFO. Inputs are packed into wide tensors (2-11KB lines) and ordered
by need: b1 and the b=0 slice of the dec side first, W2 (first needed
~24us in) at the back of the scalar ring's FIFO.

Phase 1 is split by batch: only dech[b=0] is computed up front (16
fast 100-col matmuls); the 300-col dec matmuls for b=1..3 are emitted
as PE filler between the first chunk groups, exactly where the b0
build chain (add -> tanh) would otherwise stall the PE.

Phase 2 emission is software-pipelined by one chunk — build(i+1)
[broadcast-add + tanh], then matmuls(i), then psum copies(i) — so the
in-order DVE/ACT queues always hold ready work ahead of the
PE-dependent psum copies (avoids head-of-line blocking).  Chunks are
up to 5 t's (500-row streams; matmul moving size is ISA-limited to
512 = one PSUM bank).

Engine assignment per chunk: DVE broadcast-adds k0..6 (~110ns per
100-elem line), GpSimd only k7 (it measures ~0.8us per line), ACT does
both tanhs and 2 psum copies, DVE the other 2 (GpSimd cannot read
PSUM). Output is written bf16 (halves out-DMA); b2 and the fp32
upcast happen on the host.
"""

from contextlib import ExitStack

import ml_dtypes
import numpy as np

import concourse.bacc as bacc
import concourse.bass as bass
import concourse.mybir as mybir
import concourse.tile as tile
from concourse.bass_utils import run_bass_kernel_spmd

F32 = mybir.dt.float32
BF16 = mybir.dt.bfloat16

B, T, U, D, H, O = 4, 200, 100, 512, 1024, 512
NCORES = 8
TLOC = T // NCORES            # 25 t's per core
PAIRS = B * TLOC              # 100 (b,t) pairs per core
BU = B * U                    # 400
ROWS = PAIRS * U              # 10000 output rows per core
DK = D // 128                 # 4 contraction chunks for phase 1
HK = H // 128                 # 8 h chunks
KA = 7                        # k chunks in the A half (k=0..6, DVE)
KB = HK - KA                  # 1 k chunk in the B half (k=7, GpSimd)
CHMAX = 500                   # max rows per phase-2 chunk
U3 = 3 * U                    # 300 dec cols for b=1..3

ENC_W = DK * PAIRS            # 400
DEC0_W = DK * U               # 400: b=0 slice of decT
DEC3_W = DK * U3              # 1200: b=1..3 slice
W1H = (HK // 2) * 512         # 2048: k-major half of a W1 side

_CACHE = {}


def _chunks():
    sizes_by_b = [
        [1, 2, 3, 4, 5, 5, 5],
        [5] * 5,
        [5] * 5,
        [5, 5, 5, 5, 4, 1],
    ]
    out = []
    for b, sizes in enumerate(sizes_by_b):
        t0 = 0
        for tch in sizes:
            out.append((b, t0, tch))
            t0 += tch
        assert t0 == TLOC
    return out


def _build():
    nc = bacc.Bacc("TRN2", target_bir_lowering=False, debug=False,
                   num_devices=NCORES)
    # k-major W1 halves: col = (k % 4)*512 + dk*128 + j
    inSa = nc.dram_tensor("inSa", [128, ENC_W + W1H], BF16,
                          kind="ExternalInput")   # encT | w1e k0..3
    inSb = nc.dram_tensor("inSb", [128, W1H], BF16,
                          kind="ExternalInput")   # w1e k4..7
    inCa = nc.dram_tensor("inCa", [128, DEC0_W + W1H], BF16,
                          kind="ExternalInput")   # decT b0 | w1d k0..3
    inCb = nc.dram_tensor("inCb", [128, W1H], BF16,
                          kind="ExternalInput")   # w1d k4..7
    dec3 = nc.dram_tensor("dec3", [128, DEC3_W], BF16,
                          kind="ExternalInput")   # decT b1..3
    w2T = nc.dram_tensor("w2T", [128, HK * O], BF16, kind="ExternalInput")
    b1r = nc.dram_tensor("b1r", [128, HK], F32, kind="ExternalInput")
    out = nc.dram_tensor("out", [O, ROWS], BF16, kind="ExternalOutput")

    with tile.TileContext(nc) as tc, ExitStack() as ctx:
        consts = ctx.enter_context(tc.tile_pool(name="consts", bufs=1))
        spoolA = ctx.enter_context(tc.tile_pool(name="spoolA", bufs=3))
        spoolB = ctx.enter_context(tc.tile_pool(name="spoolB", bufs=3))
        opool = ctx.enter_context(tc.tile_pool(name="opool", bufs=8))
        psB = ctx.enter_context(tc.tile_pool(name="psB", bufs=8, space="PSUM"))

        inSa_s = consts.tile([128, ENC_W + W1H], BF16)
        inSb_s = consts.tile([128, W1H], BF16)
        inCa_s = consts.tile([128, DEC0_W + W1H], BF16)
        inCb_s = consts.tile([128, W1H], BF16)
        dec3_s = consts.tile([128, DEC3_W], BF16)
        w2_s = consts.tile([128, HK * O], BF16)
        b1_s = consts.tile([128, HK], F32)
        nc.scalar.dma_start(b1_s[:], b1r[:])
        nc.sync.dma_start(inSa_s[:], inSa[:])
        nc.scalar.dma_start(inCa_s[:], inCa[:])
        nc.sync.dma_start(inSb_s[:], inSb[:])
        nc.scalar.dma_start(inCb_s[:], inCb[:])
        nc.scalar.dma_start(dec3_s[:], dec3[:])
        nc.scalar.dma_start(w2_s[:], w2T[:])
        encT_s = inSa_s[:, :ENC_W]
        dec0_s = inCa_s[:, :DEC0_W]

        def w1e_blk(k, dk):
            if k < 4:
                return inSa_s[:, ENC_W + k * 512 + dk * 128:
                              ENC_W + k * 512 + (dk + 1) * 128]
            return inSb_s[:, (k - 4) * 512 + dk * 128:
                          (k - 4) * 512 + (dk + 1) * 128]

        def w1d_blk(k, dk):
            if k < 4:
                return inCa_s[:, DEC0_W + k * 512 + dk * 128:
                              DEC0_W + k * 512 + (dk + 1) * 128]
            return inCb_s[:, (k - 4) * 512 + dk * 128:
                          (k - 4) * 512 + (dk + 1) * 128]

        # ---- phase 1 tiles ----
        ench_t = {"A": consts.tile([128, KA * PAIRS], BF16, name="enchA"),
                  "B": consts.tile([128, KB * PAIRS], BF16, name="enchB")}
        dech0 = {"A": consts.tile([128, KA * U], BF16, name="dech0A"),
                 "B": consts.tile([128, KB * U], BF16, name="dech0B")}
        dech3 = {"A": consts.tile([128, KA * U3], BF16, name="dech3A"),
                 "B": consts.tile([128, KB * U3], BF16, name="dech3B")}

        def halfslot(k):
            return ("A", k) if k < KA else ("B", k - KA)

        def p1_enc(k):
            pe = psB.tile([128, 512], F32, tag="psB", name="pe")[:, :PAIRS]
            for dk in range(DK):
                nc.tensor.matmul(
                    pe[:], lhsT=w1e_blk(k, dk),
                    rhs=encT_s[:, dk * PAIRS:(dk + 1) * PAIRS],
                    start=(dk == 0), stop=(dk == DK - 1),
                )
            hf, kk = halfslot(k)
            nc.scalar.activation(
                ench_t[hf][:, kk * PAIRS:(kk + 1) * PAIRS], pe[:],
                mybir.ActivationFunctionType.Identity, bias=b1_s[:, k:k + 1])

        def p1_dec0(k):
            pd = psB.tile([128, 512], F32, tag="psB", name="pd")[:, :U]
            for dk in range(DK):
                nc.tensor.matmul(
                    pd[:], lhsT=w1d_blk(k, dk),
                    rhs=dec0_s[:, dk * U:(dk + 1) * U],
                    start=(dk == 0), stop=(dk == DK - 1),
                )
            hf, kk = halfslot(k)
            # all dech copies on DVE: ACT's early queue must stay clear
            # for the first tanhs (its static order can't be trusted to
            # prioritize them past queued copies)
            nc.vector.tensor_copy(dech0[hf][:, kk * U:(kk + 1) * U], pd[:])

        def p1_dec3(k):
            pd = psB.tile([128, 512], F32, tag="psB", name="pd3")[:, :U3]
            for dk in range(DK):
                nc.tensor.matmul(
                    pd[:], lhsT=w1d_blk(k, dk),
                    rhs=dec3_s[:, dk * U3:(dk + 1) * U3],
                    start=(dk == 0), stop=(dk == DK - 1),
                )
            hf, kk = halfslot(k)
            nc.vector.tensor_copy(dech3[hf][:, kk * U3:(kk + 1) * U3], pd[:])

        # ordered by DMA arrival: inSa, inSb, inCa, inCb
        for k in range(HK):
            p1_enc(k)
        for k in range(HK):
            p1_dec0(k)

        # ---- phase 2, software-pipelined by one chunk ----
        chunks = _chunks()
        s_tiles = [None] * len(chunks)
        ps_tiles = [None] * len(chunks)

        def build(i):
            b, t0c, tch = chunks[i]
            rows_c = tch * U
            s_t = {"A": spoolA.tile([128, KA * CHMAX], BF16, tag="sA",
                                    name="sA"),
                   "B": spoolB.tile([128, KB * CHMAX], BF16, tag="sB",
                                    name="sB")}
            s_tiles[i] = s_t
            for hf, nk, eng in (("B", KB, nc.gpsimd), ("A", KA, nc.vector)):
                if b == 0:
                    dech_ap = dech0[hf][:].rearrange(
                        "p (k u) -> p k u", k=nk)
                else:
                    dech_ap = dech3[hf][:].rearrange(
                        "p (k bu) -> p k bu", k=nk)[
                            :, :, (b - 1) * U:b * U]
                dech_ap = dech_ap.rearrange("p k (a u) -> p k a u", a=1)
                c0 = b * TLOC + t0c
                ench_ap = ench_t[hf][:].rearrange(
                    "p (k c) -> p k c", k=nk)[:, :, c0:c0 + tch]
                ench_ap = ench_ap.rearrange("p k (t a) -> p k t a", a=1)
                bc_d, bc_e = bass.broadcast_tensor_aps(dech_ap, ench_ap)
                outap = s_t[hf][:, :nk * rows_c].rearrange(
                    "p (k t u) -> p k t u", k=nk, t=tch)
                eng.tensor_tensor(outap, bc_d, bc_e, mybir.AluOpType.add)
                nc.scalar.activation(s_t[hf][:, :nk * rows_c],
                                     s_t[hf][:, :nk * rows_c],
                                     mybir.ActivationFunctionType.Tanh)

        def mms(i):
            b, t0c, tch = chunks[i]
            rows_c = tch * U
            s_t = s_tiles[i]
            ps = []
            for oc in range(O // 128):
                p = psB.tile([128, 512], F32, tag="psB",
                             name="p")[:, :rows_c]
                ps.append(p)
                for k in range(KA):
                    nc.tensor.matmul(
                        p[:],
                        lhsT=w2_s[:, k * O + oc * 128: k * O + (oc + 1) * 128],
                        rhs=s_t["A"][:, k * rows_c:(k + 1) * rows_c],
                        start=(k == 0), stop=False,
                    )
                k = KA
                nc.tensor.matmul(
                    p[:],
                    lhsT=w2_s[:, k * O + oc * 128: k * O + (oc + 1) * 128],
                    rhs=s_t["B"][:, :rows_c],
                    start=False, stop=True,
                )
            ps_tiles[i] = ps

        def copies(i):
            b, t0c, tch = chunks[i]
            rows_c = tch * U
            row0 = b * (TLOC * U) + t0c * U
            ps = ps_tiles[i]
            for oc in range(O // 128):
                ot = opool.tile([128, CHMAX], BF16, tag="ot",
                                name="ot")[:, :rows_c]
                # gpsimd cannot access PSUM; split copies ACT/DVE
                if oc < 2:
                    nc.scalar.activation(ot[:], ps[oc][:],
                                         mybir.ActivationFunctionType.Copy)
                else:
                    nc.vector.tensor_copy(ot[:], ps[oc][:])
                ring = nc.sync if oc % 2 == 0 else nc.scalar
                ring.dma_start(
                    out[oc * 128:(oc + 1) * 128, row0:row0 + rows_c], ot[:])

        # prologue: the dec matmuls for b=1..3 are PE filler while the
        # b0 build chain (DVE add -> ACT tanh) catches up
        build(0)
        build(1)
        mms(0)
        copies(0)
        build(2)
        for k in range(4):
            p1_dec3(k)
        mms(1)
        copies(1)
        build(3)
        for k in range(4, HK):
            p1_dec3(k)
        mms(2)
        copies(2)
        for i in range(3, len(chunks)):
            if i + 1 < len(chunks):
                build(i + 1)
            mms(i)
            copies(i)
    nc.compile()
    return nc


def _chunk128(a):
    # [n*128, w] -> [128, n*w]: partition p holds row k*128+p of chunk k
    n = a.shape[0] // 128
    return np.ascontiguousarray(
        a.reshape(n, 128, a.shape[1]).transpose(1, 0, 2).reshape(128, -1))


def _bf16(a):
    return np.ascontiguousarray(a).astype(ml_dtypes.bfloat16)


def _kmajor(w1T):
    # [128, dk-major (DK x H)] -> [128, k-major (HK x DK x 128)]
    return np.ascontiguousarray(
        w1T.reshape(128, DK, HK, 128).transpose(0, 2, 1, 3).reshape(128, -1))


def kernel(enc_state, dec_state, W1, b1, W2, b2, _trace=False):
    enc_state = np.ascontiguousarray(enc_state, dtype=np.float32)
    dec_state = np.ascontiguousarray(dec_state, dtype=np.float32)
    W1 = np.asarray(W1, dtype=np.float32)
    b1 = np.asarray(b1, dtype=np.float32)
    W2 = np.asarray(W2, dtype=np.float32)
    b2 = np.asarray(b2, dtype=np.float32)

    if "nc" not in _CACHE:
        _CACHE["nc"] = _build()
    nc = _CACHE["nc"]

    decT = _chunk128(dec_state.reshape(B * U, D).T)     # [128, DK*BU]
    dec0 = np.concatenate(
        [decT[:, dk * BU: dk * BU + U] for dk in range(DK)], axis=1)
    dec3 = np.concatenate(
        [decT[:, dk * BU + U:(dk + 1) * BU] for dk in range(DK)], axis=1)
    w1e_km = _kmajor(_chunk128(W1[:, :D].T))
    w1d_km = _kmajor(_chunk128(W1[:, D:].T))
    w2T = _bf16(_chunk128(W2.T))
    b1r = np.ascontiguousarray(b1.reshape(HK, 128).T)
    inCa = _bf16(np.concatenate([dec0, w1d_km[:, :W1H]], axis=1))
    inCb = _bf16(w1d_km[:, W1H:])
    inSb = _bf16(w1e_km[:, W1H:])
    dec3 = _bf16(dec3)

    in_maps = []
    for c in range(NCORES):
        enc_c = enc_state[:, c * TLOC:(c + 1) * TLOC, :].reshape(PAIRS, D)
        encT = _chunk128(enc_c.T)
        in_maps.append({
            "inSa": _bf16(np.concatenate([encT, w1e_km[:, :W1H]], axis=1)),
            "inSb": inSb, "inCa": inCa, "inCb": inCb, "dec3": dec3,
            "w2T": w2T, "b1r": b1r,
        })

    res = run_bass_kernel_spmd(nc, in_maps, list(range(NCORES)), trace=_trace)
    out = np.empty((B, T, U, O), dtype=np.float32)
    for c in range(NCORES):
        out[:, c * TLOC:(c + 1) * TLOC] = (
            res.results[c]["out"].astype(np.float32).T.reshape(
                B, TLOC, U, O))
    out += b2
    if _trace:
        kernel.last_results = res
    return out
